# revision 7
# baseline (speedup 1.0000x reference)
"""Causal self-attention kernel for Trainium2, sharded over 8 NeuronCores.

Sharding: data-parallel over batch (B=4) x tensor-parallel over heads
(2 groups of 8 heads).  Core c handles batch c//2, head-group c%2.
Each core computes qkv for its head slice, full causal attention for its
8 heads, and a row-parallel partial projection; the host sums the two
partial projections per batch (the TP all-reduce) and adds the output
bias (b_proj + b_v @ W_proj, exact because softmax rows sum to 1).

Key structure vs the straightforward version:
  - QK^T head PAIRS run as row-tiled concurrent matmuls: heads 2m/2m+1
    live at partitions 0-63 / 64-127 of feature tile m, so their K=64
    matmuls occupy disjoint PE row-groups and overlap (~2x).
  - S^T, exp and att@V are restricted to the causal query window of
    each key tile.
  - Softmax denominators come from a ones-column prepended to V; the
    per-query reciprocal row is broadcast across 64 partitions with a
    rank-1 fp32r matmul into the spare rows of the att@V PSUM bank (no
    DRAM round trip).
  - qkv matmuls of chunk n+1 and the projection of chunk n-1 are
    interleaved one-per-attention-step into the PE stream, filling the
    stalls left while the Scalar engine computes exp.
  - x transposes ride the Sync queue; weight DMAs are spread across
    engine queues so the first matmul starts early.
"""

import sys

for _p in ("/opt/trn_rl_repo", "/root/.axon_site/_ro/trn_rl_repo"):
    if _p not in sys.path:
        sys.path.insert(0, _p)

import ml_dtypes
import numpy as np

import concourse.bass as bass
import concourse.mybir as mybir
import concourse.tile as tile
from concourse import bacc, bass_utils

F32 = mybir.dt.float32
F32R = mybir.dt.float32r
BF16 = mybir.dt.bfloat16
AF = mybir.ActivationFunctionType

B, T, D = 4, 2048, 1024
H, HD = 16, 64
HG = 2                      # head groups (tensor-parallel factor)
H_LOC = H // HG             # 8 heads per core
DH = H_LOC * HD             # 512 local qkv width
N_CORES = 8
SCALE = 1.0 / np.sqrt(HD)


def r(ap):
    return ap.bitcast(F32R)


def build_attention(t_len=T, d_model=D, dh=DH):
    KC = d_model // 128          # contraction chunks for qkv
    NT = t_len // 128            # token tiles
    NQ = t_len // 512            # token chunks (= query chunks)
    NF = dh // 128               # feature tiles of q/k (= head pairs)
    NH = dh // HD                # local heads
    KP = dh // 128               # contraction chunks for proj
    ND = d_model // 512          # output column chunks

    nc = bacc.Bacc("TRN2", target_bir_lowering=False, debug=False,
                   num_devices=N_CORES)

    x = nc.dram_tensor("x", [t_len, d_model], BF16, kind="ExternalInput")
    wq = nc.dram_tensor("wq", [d_model, dh], BF16, kind="ExternalInput")
    wk = nc.dram_tensor("wk", [d_model, dh], BF16, kind="ExternalInput")
    wv = nc.dram_tensor("wv", [d_model, dh], BF16, kind="ExternalInput")
    bqs = nc.dram_tensor("bqs", [dh], F32, kind="ExternalInput")  # pre-scaled
    bk = nc.dram_tensor("bk", [dh], F32, kind="ExternalInput")
    wp = nc.dram_tensor("wp", [dh, d_model], F32R, kind="ExternalInput")
    out = nc.dram_tensor("out", [t_len, d_model], F32, kind="ExternalOutput")

    with tile.TileContext(nc) as tc:
        with (
            tc.tile_pool(name="singles", bufs=1) as singles,
            tc.tile_pool(name="persist", bufs=1) as persist,
            tc.tile_pool(name="xt", bufs=2) as pool_xt,
            tc.tile_pool(name="st", bufs=6) as pool_st,
            tc.tile_pool(name="rr", bufs=4) as pool_rr,
            tc.tile_pool(name="ostg", bufs=4) as pool_ostg,
            tc.tile_pool(name="ps_mm", bufs=2, space="PSUM") as ps_mm,
            tc.tile_pool(name="ps_st", bufs=2, space="PSUM") as ps_st,
            tc.tile_pool(name="pot", bufs=2, space="PSUM") as pool_pot,
        ):
            # persistent activations
            qT = persist.tile([128, NF, t_len], BF16, tag="qT")  # [feat, tok]
            kT = persist.tile([128, NF, t_len], BF16, tag="kT")
            vaug = persist.tile([128, NT, NH, HD + 1], BF16, tag="vaug")
            oT = persist.tile([128, NF, t_len], F32R, tag="oT")
            ones_r = singles.tile([1, 64], BF16, tag="ones")

            # x chunk 0 transpose first: it gates the first matmul.
            xts = [None] * NQ

            def emit_xt(n):
                xts[n] = pool_xt.tile([128, KC, 512], BF16, tag="xt",
                                      name=f"xt{n}")
                for dc in range(KC):
                    nc.sync.dma_start_transpose(
                        xts[n][:, dc, :],
                        x[n * 512:(n + 1) * 512, dc * 128:(dc + 1) * 128])

            emit_xt(0)

            # weight/bias loads spread across engine queues so they
            # overlap each other and the x transpose.
            wq_sb = singles.tile([128, KC, dh], BF16, tag="wq")
            nc.gpsimd.dma_start(wq_sb, wq.rearrange("(c p) n -> p c n", p=128))
            wk_sb = singles.tile([128, KC, dh], BF16, tag="wk")
            nc.scalar.dma_start(wk_sb, wk.rearrange("(c p) n -> p c n", p=128))
            wv_sb = singles.tile([128, KC, dh], BF16, tag="wv")
            nc.gpsimd.dma_start(wv_sb, wv.rearrange("(c p) n -> p c n", p=128))
            bqs_sb = singles.tile([128, NF], F32, tag="bqs")
            nc.scalar.dma_start(bqs_sb, bqs.rearrange("(f p) -> p f", p=128))
            bk_sb = singles.tile([128, NF], F32, tag="bk")
            nc.scalar.dma_start(bk_sb, bk.rearrange("(f p) -> p f", p=128))
            wp_sb = singles.tile([128, KP, d_model], F32R, tag="wp")
            nc.scalar.dma_start(wp_sb, wp.rearrange("(c p) n -> p c n", p=128))

            nc.vector.memset(vaug[:, :, :, HD:HD + 1], 1.0)
            nc.vector.memset(ones_r[:, :], 1.0)

            # ---------- filler streams: one PE matmul per closure ----------
            def qkv_work(n):
                """q/k/v matmuls (+drains) for chunk n as a closure stream."""
                xt = xts[n]
                for f in range(NF):
                    for which, w_sb, bias, dstT in (
                        ("q", wq_sb, bqs_sb, qT),
                        ("k", wk_sb, bk_sb, kT),
                    ):
                        pqk = [None]

                        def em(c, which=which, w_sb=w_sb, bias=bias,
                               dstT=dstT, f=f, pqk=pqk):
                            if c == 0:
                                pqk[0] = ps_mm.tile([128, 512], F32, tag="mm",
                                                    name=f"p_{which}{f}_{n}")
                            nc.tensor.matmul(
                                pqk[0][:, :],
                                lhsT=w_sb[:, c, f * 128:(f + 1) * 128],
                                rhs=xt[:, c, :],
                                start=(c == 0), stop=(c == KC - 1))
                            if c == KC - 1:
                                nc.vector.tensor_scalar_add(
                                    out=dstT[:, f, n * 512:(n + 1) * 512],
                                    in0=pqk[0][:, :],
                                    scalar1=bias[:, f:f + 1])

                        for c in range(KC):
                            yield lambda c=c, em=em: em(c)
                for tt in range(4):
                    t = 4 * n + tt
                    pv = [None]

                    def emv(c, t=t, tt=tt, pv=pv):
                        if c == 0:
                            pv[0] = ps_mm.tile([128, dh], F32, tag="mm",
                                               name=f"pv{t}")
                        nc.tensor.matmul(
                            pv[0][:, :],
                            lhsT=xt[:, c, tt * 128:(tt + 1) * 128],
                            rhs=wv_sb[:, c, :],
                            start=(c == 0), stop=(c == KC - 1))
                        if c == KC - 1:
                            nc.vector.tensor_copy(
                                vaug[:, t, :, 0:HD],
                                pv[0].rearrange("p (h e) -> p h e", e=HD))

                    for c in range(KC):
                        yield lambda c=c, emv=emv: emv(c)

            def proj_work(n):
                """Projection matmuls (+drains) for token chunk n."""
                for tt in range(4):
                    t = 4 * n + tt
                    for nn in range(ND):
                        pd = [None]

                        def emp(c, t=t, nn=nn, pd=pd):
                            if c == 0:
                                pd[0] = ps_mm.tile([128, 512], F32, tag="mm",
                                                   name=f"pd{t}_{nn}")
                            nc.tensor.matmul(
                                pd[0][:, :],
                                lhsT=r(oT[:, c, t * 128:(t + 1) * 128]),
                                rhs=r(wp_sb[:, c, nn * 512:(nn + 1) * 512]),
                                start=(c == 0), stop=(c == KP - 1))
                            if c == KP - 1:
                                ostg = pool_ostg.tile(
                                    [128, 512], F32, tag="ostg",
                                    name=f"ostg{t}_{nn}")
                                nc.vector.tensor_copy(ostg[:, :], pd[0][:, :])
                                nc.sync.dma_start(
                                    out[t * 128:(t + 1) * 128,
                                        nn * 512:(nn + 1) * 512],
                                    ostg[:, :])

                        for c in range(KP):
                            yield lambda c=c, emp=emp: emp(c)

            # consume chunk 0's qkv up front (nothing to overlap it with)
            for em in qkv_work(0):
                em()

            # ---------------- main software-pipelined loop -----------------
            for n in range(NQ):
                fillers = []
                if n + 1 < NQ:
                    emit_xt(n + 1)
                    fillers.extend(qkv_work(n + 1))
                if n - 1 >= 0:
                    fillers.extend(proj_work(n - 1))
                fillers = iter(fillers)

                ntk = 4 * n + 4
                for m in range(NF):        # head pair (2m, 2m+1)
                    potA = pool_pot.tile([128, 512], F32, tag="pot",
                                         name=f"potA{n}_{m}")
                    potB = pool_pot.tile([128, 512], F32, tag="pot",
                                         name=f"potB{n}_{m}")

                    def emit_S(ti):
                        w0 = max(0, ti * 128 - n * 512)
                        pst = ps_st.tile([128, 2, 512], F32, tag="st",
                                         name=f"pst{n}_{m}_{ti}")
                        sts = pool_st.tile([128, 2, 512], BF16, tag="st",
                                           name=f"st{n}_{m}_{ti}")
                        for j, rb in ((0, 0), (1, 64)):
                            nc.tensor.matmul(
                                pst[:, j, w0:],
                                lhsT=kT[rb:rb + 64, m,
                                        ti * 128:(ti + 1) * 128],
                                rhs=qT[rb:rb + 64, m,
                                       n * 512 + w0:(n + 1) * 512],
                                start=True, stop=True)
                        nc.scalar.activation(sts[:, :, w0:], pst[:, :, w0:],
                                             AF.Exp)
                        if ti >= 4 * n:   # diagonal: mask q < k after exp
                            nc.gpsimd.affine_select(
                                out=sts[:, :, w0:w0 + 128],
                                in_=sts[:, :, w0:w0 + 128],
                                compare_op=mybir.AluOpType.is_ge,
                                fill=0.0,
                                base=0,
                                channel_multiplier=-1,
                                pattern=[[0, 2], [1, 128]])
                        return sts, w0

                    def emit_AV(ti, sts, w0):
                        for j, pot, h in ((0, potA, 2 * m), (1, potB, 2 * m + 1)):
                            nc.tensor.matmul(
                                pot[0:HD + 1, w0:],
                                lhsT=vaug[:, ti, h, 0:HD + 1],
                                rhs=sts[:, j, w0:],
                                start=(ti == 0), stop=(ti == ntk - 1))

                    prev = emit_S(0)
                    for ti in range(ntk):
                        nxt = emit_S(ti + 1) if ti + 1 < ntk else None
                        f = next(fillers, None)
                        if f is not None:
                            f()
                        emit_AV(ti, *prev)
                        prev = nxt

                    # drain: evict features, reciprocal of the ones-row
                    # denominator, rank-1 broadcast into the spare PSUM
                    # rows, normalize in place.
                    for pot, rb in ((potA, 0), (potB, 64)):
                        dst = oT[rb:rb + 64, m, n * 512:(n + 1) * 512]
                        nc.vector.tensor_copy(dst, pot[0:HD, :])
                        rr = pool_rr.tile([1, 512], BF16, tag="rr",
                                          name=f"rr{n}_{m}_{rb}")
                        with nc.allow_low_precision(
                                reason="f32r softmax denom broadcast"):
                            nc.vector.reciprocal(rr[:, :], pot[HD:HD + 1, :])
                        nc.tensor.matmul(
                            pot[64:128, :],
                            lhsT=ones_r[0:1, 0:64],
                            rhs=rr[:, :],
                            start=True, stop=True)
                        nc.vector.tensor_mul(dst, dst.bitcast(F32),
                                             pot[64:128, :])

                # flush leftover fillers (rest of next chunk's qkv + proj)
                for f in fillers:
                    f()

            # tail: projection of the last chunk
            for em in proj_work(NQ - 1):
                em()

    nc.compile()
    return nc


_NC_CACHE = {}


def _get_nc():
    if "nc" not in _NC_CACHE:
        _NC_CACHE["nc"] = build_attention()
    return _NC_CACHE["nc"]


def shard_inputs(x, W_qkv, b_qkv, W_proj):
    bf = ml_dtypes.bfloat16
    in_maps = []
    for c in range(N_CORES):
        b, hg = divmod(c, HG)
        cs = slice(hg * DH, (hg + 1) * DH)
        m = {
            "x": np.ascontiguousarray(x[b]).astype(bf),
            "wq": (np.ascontiguousarray(W_qkv[:, 0 * D:1 * D][:, cs])
                   * np.float32(SCALE)).astype(bf),
            "wk": np.ascontiguousarray(W_qkv[:, 1 * D:2 * D][:, cs]).astype(bf),
            "wv": np.ascontiguousarray(W_qkv[:, 2 * D:3 * D][:, cs]).astype(bf),
            "bqs": np.ascontiguousarray(b_qkv[0 * D:1 * D][cs]) * np.float32(SCALE),
            "bk": np.ascontiguousarray(b_qkv[1 * D:2 * D][cs]),
            "wp": np.ascontiguousarray(W_proj[cs, :]),
        }
        in_maps.append(m)
    return in_maps


def kernel(x, W_qkv, b_qkv, W_proj, b_proj, _trace=False, _trace_kwargs=None):
    x = np.asarray(x, dtype=np.float32)
    W_qkv = np.asarray(W_qkv, dtype=np.float32)
    b_qkv = np.asarray(b_qkv, dtype=np.float32)
    W_proj = np.asarray(W_proj, dtype=np.float32)
    b_proj = np.asarray(b_proj, dtype=np.float32)

    nc = _get_nc()
    in_maps = shard_inputs(x, W_qkv, b_qkv, W_proj)
    res = bass_utils.run_bass_kernel_spmd(
        nc, in_maps, core_ids=list(range(N_CORES)),
        trace=_trace, **(_trace_kwargs or {}))

    # softmax rows sum to 1, so att@(v + b_v) == att@v + b_v exactly:
    # fold the v bias through the projection on the host.
    bias_full = (b_proj + b_qkv[2 * D:3 * D] @ W_proj).astype(np.float32)
    out = np.empty((B, T, D), dtype=np.float32)
    for b in range(B):
        acc = res.results[HG * b]["out"].astype(np.float32)
        for hg in range(1, HG):
            acc = acc + res.results[HG * b + hg]["out"]
        out[b] = acc + bias_full[None, :]
    if _trace:
        return out, res
    return out


# revision 9
# speedup vs baseline: 1.4379x; 1.4379x over previous
"""Causal self-attention kernel for Trainium2, sharded over 8 NeuronCores.

Sharding: data-parallel over batch (B=4) x tensor-parallel over heads
(2 groups of 8 heads).  Core c handles batch c//2, head-group c%2.
Each core computes qkv for its head slice, full causal attention for its
8 heads, and a row-parallel partial projection; the host sums the two
partial projections per batch (the TP all-reduce) and adds the output
bias (b_proj + b_v @ W_proj, exact because softmax rows sum to 1).

Key structure vs the straightforward version:
  - QK^T head PAIRS run as row-tiled concurrent matmuls: heads 2m/2m+1
    live at partitions 0-63 / 64-127 of feature tile m, so their K=64
    matmuls occupy disjoint PE row-groups and overlap (~2x).
  - S^T, exp and att@V are restricted to the causal query window of
    each key tile.
  - Softmax denominators come from a ones-column prepended to V; the
    per-query reciprocal row is broadcast across 64 partitions with a
    rank-1 fp32r matmul into the spare rows of the att@V PSUM bank (no
    DRAM round trip).
  - qkv matmuls of chunk n+1 and the projection of chunk n-1 are
    interleaved one-per-attention-step into the PE stream, filling the
    stalls left while the Scalar engine computes exp.
  - x transposes ride the Sync queue; weight DMAs are spread across
    engine queues so the first matmul starts early.
"""

import sys

for _p in ("/opt/trn_rl_repo", "/root/.axon_site/_ro/trn_rl_repo"):
    if _p not in sys.path:
        sys.path.insert(0, _p)

import ml_dtypes
import numpy as np

import concourse.bass as bass
import concourse.mybir as mybir
import concourse.tile as tile
from concourse import bacc, bass_utils

F32 = mybir.dt.float32
F32R = mybir.dt.float32r
BF16 = mybir.dt.bfloat16
AF = mybir.ActivationFunctionType

B, T, D = 4, 2048, 1024
H, HD = 16, 64
HG = 2                      # head groups (tensor-parallel factor)
H_LOC = H // HG             # 8 heads per core
DH = H_LOC * HD             # 512 local qkv width
N_CORES = 8
SCALE = 1.0 / np.sqrt(HD)


def r(ap):
    return ap.bitcast(F32R)


def build_attention(t_len=T, d_model=D, dh=DH):
    KC = d_model // 128          # contraction chunks for qkv
    NT = t_len // 128            # token tiles
    NQ = t_len // 512            # token chunks (= query chunks)
    NF = dh // 128               # feature tiles of q/k (= head pairs)
    NH = dh // HD                # local heads
    KP = dh // 128               # contraction chunks for proj
    ND = d_model // 512          # output column chunks

    nc = bacc.Bacc("TRN2", target_bir_lowering=False, debug=False,
                   num_devices=N_CORES)

    x = nc.dram_tensor("x", [t_len, d_model], BF16, kind="ExternalInput")
    wq = nc.dram_tensor("wq", [d_model, dh], BF16, kind="ExternalInput")
    wk = nc.dram_tensor("wk", [d_model, dh], BF16, kind="ExternalInput")
    wv = nc.dram_tensor("wv", [d_model, dh], BF16, kind="ExternalInput")
    bqs = nc.dram_tensor("bqs", [dh], F32, kind="ExternalInput")  # pre-scaled
    bk = nc.dram_tensor("bk", [dh], F32, kind="ExternalInput")
    wp = nc.dram_tensor("wp", [dh, d_model], F32R, kind="ExternalInput")
    out = nc.dram_tensor("out", [t_len, d_model], F32, kind="ExternalOutput")

    with tile.TileContext(nc) as tc:
        with (
            tc.tile_pool(name="singles", bufs=1) as singles,
            tc.tile_pool(name="persist", bufs=1) as persist,
            tc.tile_pool(name="xt", bufs=2) as pool_xt,
            tc.tile_pool(name="st", bufs=6) as pool_st,
            tc.tile_pool(name="dn", bufs=3) as pool_dn,
            tc.tile_pool(name="dnd", bufs=4, space="DRAM") as pool_dnd,
            tc.tile_pool(name="ostg", bufs=4) as pool_ostg,
            tc.tile_pool(name="ps_mm", bufs=2, space="PSUM") as ps_mm,
            tc.tile_pool(name="ps_st", bufs=2, space="PSUM") as ps_st,
            tc.tile_pool(name="pot", bufs=2, space="PSUM") as pool_pot,
        ):
            # persistent activations
            qT = persist.tile([128, NF, t_len], BF16, tag="qT")  # [feat, tok]
            kT = persist.tile([128, NF, t_len], BF16, tag="kT")
            vaug = persist.tile([128, NT, NH, HD + 1], BF16, tag="vaug")
            oT = persist.tile([128, NF, t_len], F32R, tag="oT")

            # x chunk 0 transpose first: it gates the first matmul.
            xts = [None] * NQ

            def emit_xt(n):
                xts[n] = pool_xt.tile([128, KC, 512], BF16, tag="xt",
                                      name=f"xt{n}")
                for dc in range(KC):
                    nc.sync.dma_start_transpose(
                        xts[n][:, dc, :],
                        x[n * 512:(n + 1) * 512, dc * 128:(dc + 1) * 128])

            emit_xt(0)

            # weight/bias loads spread across engine queues so they
            # overlap each other and the x transpose.
            wq_sb = singles.tile([128, KC, dh], BF16, tag="wq")
            nc.gpsimd.dma_start(wq_sb, wq.rearrange("(c p) n -> p c n", p=128))
            wk_sb = singles.tile([128, KC, dh], BF16, tag="wk")
            nc.scalar.dma_start(wk_sb, wk.rearrange("(c p) n -> p c n", p=128))
            wv_sb = singles.tile([128, KC, dh], BF16, tag="wv")
            nc.gpsimd.dma_start(wv_sb, wv.rearrange("(c p) n -> p c n", p=128))
            bqs_sb = singles.tile([128, NF], F32, tag="bqs")
            nc.scalar.dma_start(bqs_sb, bqs.rearrange("(f p) -> p f", p=128))
            bk_sb = singles.tile([128, NF], F32, tag="bk")
            nc.scalar.dma_start(bk_sb, bk.rearrange("(f p) -> p f", p=128))
            wp_sb = singles.tile([128, KP, d_model], F32R, tag="wp")
            nc.scalar.dma_start(wp_sb, wp.rearrange("(c p) n -> p c n", p=128))

            nc.vector.memset(vaug[:, :, :, HD:HD + 1], 1.0)

            # ---------- filler streams: one PE matmul per closure ----------
            def qkv_work(n):
                """q/k/v matmuls (+drains) for chunk n as a closure stream."""
                xt = xts[n]
                for f in range(NF):
                    for which, w_sb, bias, dstT in (
                        ("q", wq_sb, bqs_sb, qT),
                        ("k", wk_sb, bk_sb, kT),
                    ):
                        pqk = [None]

                        def em(c, which=which, w_sb=w_sb, bias=bias,
                               dstT=dstT, f=f, pqk=pqk):
                            if c == 0:
                                pqk[0] = ps_mm.tile([128, 512], F32, tag="mm",
                                                    name=f"p_{which}{f}_{n}")
                            nc.tensor.matmul(
                                pqk[0][:, :],
                                lhsT=w_sb[:, c, f * 128:(f + 1) * 128],
                                rhs=xt[:, c, :],
                                start=(c == 0), stop=(c == KC - 1))
                            if c == KC - 1:
                                nc.vector.tensor_scalar_add(
                                    out=dstT[:, f, n * 512:(n + 1) * 512],
                                    in0=pqk[0][:, :],
                                    scalar1=bias[:, f:f + 1])

                        for c in range(KC):
                            yield lambda c=c, em=em: em(c)
                for tt in range(4):
                    t = 4 * n + tt
                    pv = [None]

                    def emv(c, t=t, tt=tt, pv=pv):
                        if c == 0:
                            pv[0] = ps_mm.tile([128, dh], F32, tag="mm",
                                               name=f"pv{t}")
                        nc.tensor.matmul(
                            pv[0][:, :],
                            lhsT=xt[:, c, tt * 128:(tt + 1) * 128],
                            rhs=wv_sb[:, c, :],
                            start=(c == 0), stop=(c == KC - 1))
                        if c == KC - 1:
                            nc.vector.tensor_copy(
                                vaug[:, t, :, 0:HD],
                                pv[0].rearrange("p (h e) -> p h e", e=HD))

                    for c in range(KC):
                        yield lambda c=c, emv=emv: emv(c)

            def proj_work(n):
                """Projection matmuls (+drains) for token chunk n."""
                for tt in range(4):
                    t = 4 * n + tt
                    for nn in range(ND):
                        pd = [None]

                        def emp(c, t=t, nn=nn, pd=pd):
                            if c == 0:
                                pd[0] = ps_mm.tile([128, 512], F32, tag="mm",
                                                   name=f"pd{t}_{nn}")
                            nc.tensor.matmul(
                                pd[0][:, :],
                                lhsT=r(oT[:, c, t * 128:(t + 1) * 128]),
                                rhs=r(wp_sb[:, c, nn * 512:(nn + 1) * 512]),
                                start=(c == 0), stop=(c == KP - 1))
                            if c == KP - 1:
                                ostg = pool_ostg.tile(
                                    [128, 512], F32, tag="ostg",
                                    name=f"ostg{t}_{nn}")
                                nc.vector.tensor_copy(ostg[:, :], pd[0][:, :])
                                nc.sync.dma_start(
                                    out[t * 128:(t + 1) * 128,
                                        nn * 512:(nn + 1) * 512],
                                    ostg[:, :])

                        for c in range(KP):
                            yield lambda c=c, emp=emp: emp(c)

            # consume chunk 0's qkv up front (nothing to overlap it with)
            for em in qkv_work(0):
                em()

            # ---------------- main software-pipelined loop -----------------
            for n in range(NQ):
                fillers = []
                if n + 1 < NQ:
                    emit_xt(n + 1)
                    fillers.extend(qkv_work(n + 1))
                if n - 1 >= 0:
                    fillers.extend(proj_work(n - 1))
                fillers = iter(fillers)

                ntk = 4 * n + 4
                for m in range(NF):        # head pair (2m, 2m+1)
                    potA = pool_pot.tile([128, 512], F32, tag="pot",
                                         name=f"potA{n}_{m}")
                    potB = pool_pot.tile([128, 512], F32, tag="pot",
                                         name=f"potB{n}_{m}")

                    def emit_S(ti):
                        w0 = max(0, ti * 128 - n * 512)
                        pst = ps_st.tile([128, 2, 512], F32, tag="st",
                                         name=f"pst{n}_{m}_{ti}")
                        sts = pool_st.tile([128, 2, 512], BF16, tag="st",
                                           name=f"st{n}_{m}_{ti}")
                        for j, rb in ((0, 0), (1, 64)):
                            nc.tensor.matmul(
                                pst[:, j, w0:],
                                lhsT=kT[rb:rb + 64, m,
                                        ti * 128:(ti + 1) * 128],
                                rhs=qT[rb:rb + 64, m,
                                       n * 512 + w0:(n + 1) * 512],
                                start=True, stop=True)
                        nc.scalar.activation(sts[:, :, w0:], pst[:, :, w0:],
                                             AF.Exp)
                        if ti >= 4 * n:   # diagonal: mask q < k after exp
                            nc.gpsimd.affine_select(
                                out=sts[:, :, w0:w0 + 128],
                                in_=sts[:, :, w0:w0 + 128],
                                compare_op=mybir.AluOpType.is_ge,
                                fill=0.0,
                                base=0,
                                channel_multiplier=-1,
                                pattern=[[0, 2], [1, 128]])
                        return sts, w0

                    def emit_AV(ti, sts, w0):
                        for j, pot, h in ((0, potA, 2 * m), (1, potB, 2 * m + 1)):
                            nc.tensor.matmul(
                                pot[0:HD + 1, w0:],
                                lhsT=vaug[:, ti, h, 0:HD + 1],
                                rhs=sts[:, j, w0:],
                                start=(ti == 0), stop=(ti == ntk - 1))

                    prev = emit_S(0)
                    for ti in range(ntk):
                        nxt = emit_S(ti + 1) if ti + 1 < ntk else None
                        f = next(fillers, None)
                        if f is not None:
                            f()
                        emit_AV(ti, *prev)
                        prev = nxt

                    # drain: evict features + denominator row, then
                    # reciprocal on a [128,4] partition-spread view via a
                    # DRAM bounce (a [1,512] 1-lane reciprocal costs ~4us
                    # on DVE), broadcast back, normalize.
                    for pot, rb, q1, q2 in ((potA, 0, nc.gpsimd, nc.scalar),
                                            (potB, 64, nc.scalar, nc.gpsimd)):
                        dst = oT[rb:rb + 64, m, n * 512:(n + 1) * 512]
                        nc.vector.tensor_copy(dst, pot[0:HD, :])
                        dn = pool_dn.tile([128, 512], F32, tag="dn",
                                          name=f"dn{n}_{m}_{rb}")
                        nc.vector.tensor_copy(dn[64:65, :],
                                              pot[HD:HD + 1, :])
                        dnd = pool_dnd.tile([1, 512], F32, tag="dnd",
                                            name=f"dnd{n}_{m}_{rb}")
                        q1.dma_start(dnd[:, :], dn[64:65, :])
                        dn2 = pool_dn.tile([128, 4], F32, tag="dn2",
                                           name=f"dn2{n}_{m}_{rb}")
                        q1.dma_start(
                            dn2[:, :],
                            dnd[0, :].rearrange("(p f) -> p f", p=128))
                        nc.vector.reciprocal(dn2[:, :], dn2[:, :])
                        dnd2 = pool_dnd.tile([128, 4], F32, tag="dnd2",
                                             name=f"dnd2{n}_{m}_{rb}")
                        q2.dma_start(dnd2[:, :], dn2[:, :])
                        flat = dnd2.rearrange("p f -> (p f)")
                        bcast = bass.AP(tensor=flat.tensor,
                                        offset=flat.offset,
                                        ap=[[0, 64]] + list(flat.ap))
                        q2.dma_start(dn[rb:rb + 64, :], bcast)
                        nc.vector.tensor_mul(dst, dst.bitcast(F32),
                                             dn[rb:rb + 64, :])

                # flush leftover fillers (rest of next chunk's qkv + proj)
                for f in fillers:
                    f()

            # tail: projection of the last chunk
            for em in proj_work(NQ - 1):
                em()

    nc.compile()
    return nc


_NC_CACHE = {}


def _get_nc():
    if "nc" not in _NC_CACHE:
        _NC_CACHE["nc"] = build_attention()
    return _NC_CACHE["nc"]


def shard_inputs(x, W_qkv, b_qkv, W_proj):
    bf = ml_dtypes.bfloat16
    in_maps = []
    for c in range(N_CORES):
        b, hg = divmod(c, HG)
        cs = slice(hg * DH, (hg + 1) * DH)
        m = {
            "x": np.ascontiguousarray(x[b]).astype(bf),
            "wq": (np.ascontiguousarray(W_qkv[:, 0 * D:1 * D][:, cs])
                   * np.float32(SCALE)).astype(bf),
            "wk": np.ascontiguousarray(W_qkv[:, 1 * D:2 * D][:, cs]).astype(bf),
            "wv": np.ascontiguousarray(W_qkv[:, 2 * D:3 * D][:, cs]).astype(bf),
            "bqs": np.ascontiguousarray(b_qkv[0 * D:1 * D][cs]) * np.float32(SCALE),
            "bk": np.ascontiguousarray(b_qkv[1 * D:2 * D][cs]),
            "wp": np.ascontiguousarray(W_proj[cs, :]),
        }
        in_maps.append(m)
    return in_maps


def kernel(x, W_qkv, b_qkv, W_proj, b_proj, _trace=False, _trace_kwargs=None):
    x = np.asarray(x, dtype=np.float32)
    W_qkv = np.asarray(W_qkv, dtype=np.float32)
    b_qkv = np.asarray(b_qkv, dtype=np.float32)
    W_proj = np.asarray(W_proj, dtype=np.float32)
    b_proj = np.asarray(b_proj, dtype=np.float32)

    nc = _get_nc()
    in_maps = shard_inputs(x, W_qkv, b_qkv, W_proj)
    res = bass_utils.run_bass_kernel_spmd(
        nc, in_maps, core_ids=list(range(N_CORES)),
        trace=_trace, **(_trace_kwargs or {}))

    # softmax rows sum to 1, so att@(v + b_v) == att@v + b_v exactly:
    # fold the v bias through the projection on the host.
    bias_full = (b_proj + b_qkv[2 * D:3 * D] @ W_proj).astype(np.float32)
    out = np.empty((B, T, D), dtype=np.float32)
    for b in range(B):
        acc = res.results[HG * b]["out"].astype(np.float32)
        for hg in range(1, HG):
            acc = acc + res.results[HG * b + hg]["out"]
        out[b] = acc + bias_full[None, :]
    if _trace:
        return out, res
    return out
